# revision 10
# baseline (speedup 1.0000x reference)
"""Bass/Trainium2 kernel for nn_DecoderModel (GPT-2-like, B=4 T=1024 D=1024 H=16 L=12 V=50257).

Sharding: 8 cores; core c handles batch b=c//2, parity p=c%2.
Token rows of each batch are split into 8 tiles of 128; parity p=0 owns global
q-tiles {1,3,5,7}, p=1 owns {0,2,4,6}. Residual stream lives transposed in
SBUF as [128, 8, 512]. Per layer the only cross-core exchange is a pair
AllGather of K^T and V (bf16) through DRAM bounce buffers.

v2: weight tensors are DMA'd as large strips ([128, 8, 1024] etc., 2KB+
lines) instead of [128,128] tiles -- ~20 DMA dispatches per layer instead of
~740, which un-saturates the SP sequencer (565ns/dispatch) and keeps the PE
fed.  K/V projections run first so the pair-AllGather overlaps the Q
projection.  MLP runs in 8 hidden-octants with W1/W2 strip prefetch and
SBUF-side accumulation of the W2 partials.  LM head streams 50 vocab strips
with direct PSUM->bf16 drain (bias applied on host).
"""
import os
import sys

sys.path.insert(0, "/opt/trn_rl_repo")

import numpy as np
import ml_dtypes

import concourse.bass as bass
import concourse.mybir as mybir
import concourse.tile as tile
from concourse import bacc
from concourse.bass_utils import run_bass_kernel_spmd

BF16 = mybir.dt.bfloat16
F32 = mybir.dt.float32

B, T, D, H, NL_FULL, V = 4, 1024, 1024, 16, 12, 50257
DH = D // H              # 64
DT = D // 128            # 8 d-tiles
QT = 512 // 128          # 4 q tiles per core
VPAD = 51200             # 50 * 1024
NVC = VPAD // 1024       # 50 vocab strips
LN_EPS = 1e-5
INV_SQRT_C = 1.0 / 32.0

L = int(os.environ.get("BASSK_L", str(NL_FULL)))

KT_ELEMS = D * 512       # K^T block elems (bf16) in ag buffers
V_ELEMS = 4 * H * 65 * 128   # V blocks carry a ones column per head (denom fold)


def build_nc(num_layers=L):
    nc = bacc.Bacc("TRN2", target_bir_lowering=False, debug=True)
    NLx = num_layers

    x0T = nc.declare_dram_parameter("x0T", [128, DT, 512], F32, isOutput=False)
    wq = nc.declare_dram_parameter("wq", [NLx, D, D], BF16, isOutput=False)
    wk = nc.declare_dram_parameter("wk", [NLx, D, D], BF16, isOutput=False)
    wv = nc.declare_dram_parameter("wv", [NLx, D, D], BF16, isOutput=False)
    wo = nc.declare_dram_parameter("wo", [NLx, D, D], BF16, isOutput=False)
    w1 = nc.declare_dram_parameter("w1", [NLx, D, 4 * D], BF16, isOutput=False)
    w2 = nc.declare_dram_parameter("w2", [NLx, 4 * D, D], BF16, isOutput=False)
    ln1s = nc.declare_dram_parameter("ln1s", [NLx, 128, DT], F32, isOutput=False)
    ln1b = nc.declare_dram_parameter("ln1b", [NLx, 128, DT], F32, isOutput=False)
    ln2s = nc.declare_dram_parameter("ln2s", [NLx, 128, DT], F32, isOutput=False)
    ln2b = nc.declare_dram_parameter("ln2b", [NLx, 128, DT], F32, isOutput=False)
    bo_p = nc.declare_dram_parameter("bo_p", [NLx, 128, DT], F32, isOutput=False)
    b1_p = nc.declare_dram_parameter("b1_p", [NLx, 128, 32], F32, isOutput=False)
    b2_p = nc.declare_dram_parameter("b2_p", [NLx, 128, DT], F32, isOutput=False)
    lnfs = nc.declare_dram_parameter("lnfs", [128, DT], F32, isOutput=False)
    lnfb = nc.declare_dram_parameter("lnfb", [128, DT], F32, isOutput=False)
    lmw = nc.declare_dram_parameter("lmw", [D, VPAD], BF16, isOutput=False)
    masks = nc.declare_dram_parameter("masks", [2, 128, 128], BF16, isOutput=False)
    out = nc.declare_dram_parameter("out", [512, VPAD], BF16, isOutput=True)

    agk_in = [nc.dram_tensor(f"agk_in{i}", [KT_ELEMS], BF16) for i in range(2)]
    agk_out = [nc.dram_tensor(f"agk_out{i}", [2 * KT_ELEMS], BF16) for i in range(2)]
    agv_in = [nc.dram_tensor(f"agv_in{i}", [V_ELEMS], BF16) for i in range(2)]
    agv_out = [nc.dram_tensor(f"agv_out{i}", [2 * V_ELEMS], BF16) for i in range(2)]
    groups = [[0, 1], [2, 3], [4, 5], [6, 7]]

    from contextlib import ExitStack
    with tile.TileContext(nc) as tc, ExitStack() as es:
        const = es.enter_context(tc.tile_pool(name="const", bufs=1))
        act32 = es.enter_context(tc.tile_pool(name="act32", bufs=2))
        lnt = es.enter_context(tc.tile_pool(name="lnt", bufs=2))
        nbfp = es.enter_context(tc.tile_pool(name="nbfp", bufs=2))
        proj = es.enter_context(tc.tile_pool(name="proj", bufs=3))
        bigp = es.enter_context(tc.tile_pool(name="bigp", bufs=1))
        wpool = es.enter_context(tc.tile_pool(name="wpool", bufs=2))
        mwp = es.enter_context(tc.tile_pool(name="mwp", bufs=3))
        htp = es.enter_context(tc.tile_pool(name="htp", bufs=2))
        stp = es.enter_context(tc.tile_pool(name="stp", bufs=2))
        ev = es.enter_context(tc.tile_pool(name="ev", bufs=2))
        otp = es.enter_context(tc.tile_pool(name="otp", bufs=2))
        small = es.enter_context(tc.tile_pool(name="small", bufs=1))

        ones_bf = const.tile([128, 1], BF16)
        nc.vector.memset(ones_bf[:], 1.0)
        ones_f = const.tile([1, 128], F32)
        nc.vector.memset(ones_f[:], 1.0)
        eps_t = const.tile([1, 1], F32)
        nc.vector.memset(eps_t[:], LN_EPS)
        mask_t = const.tile([128, 2, 128], BF16)
        nc.sync.dma_start(mask_t[:], masks.rearrange("m k q -> k m q"))
        lnf_s_t = const.tile([128, DT], F32)
        nc.sync.dma_start(lnf_s_t[:], lnfs[:])
        lnf_b_t = const.tile([128, DT], F32)
        nc.sync.dma_start(lnf_b_t[:], lnfb[:])

        xT = act32.tile([128, DT, 512], F32, name="xT")
        nc.sync.dma_start(xT[:], x0T[:])

        def layernorm(x_in, s_dram, b_dram):
            """x_in: [128, DT, 512] f32 -> n_bf [128, DT, 512] bf16."""
            if s_dram is not None:
                s_t = small.tile([128, DT], F32, name="lns")
                nc.sync.dma_start(s_t[:], s_dram)
                b_t = small.tile([128, DT], F32, name="lnb")
                nc.sync.dma_start(b_t[:], b_dram)
            else:
                s_t, b_t = lnf_s_t, lnf_b_t
            with tc.tile_pool(name="lnp", bufs=2, space="PSUM") as lnp:
                ps1 = lnp.tile([1, 512], F32, name="ps")
                ps2 = lnp.tile([1, 512], F32, name="ps")
                for dt_i in range(DT):
                    xb = lnt.tile([128, 512], BF16, name="xb")
                    nc.scalar.copy(xb[:], x_in[:, dt_i])
                    sq = lnt.tile([128, 512], BF16, name="sq")
                    nc.scalar.square(sq[:], x_in[:, dt_i])
                    nc.tensor.matmul(ps1[:], ones_bf[:], xb[:],
                                     start=(dt_i == 0), stop=(dt_i == DT - 1))
                    nc.tensor.matmul(ps2[:], ones_bf[:], sq[:],
                                     start=(dt_i == 0), stop=(dt_i == DT - 1))
                mu = small.tile([1, 512], F32, name="mu")
                nc.vector.tensor_scalar_mul(mu[:], ps1[:], 1.0 / D)
                var = small.tile([1, 512], F32, name="var")
                nc.vector.tensor_scalar_mul(var[:], ps2[:], 1.0 / D)
                musq = small.tile([1, 512], F32, name="musq")
                nc.vector.tensor_mul(musq[:], mu[:], mu[:])
                nc.vector.tensor_sub(var[:], var[:], musq[:])
                sd = small.tile([1, 512], F32, name="sd")
                nc.scalar.activation(sd[:], var[:],
                                     mybir.ActivationFunctionType.Sqrt,
                                     bias=eps_t[:])
                rstd = small.tile([1, 512], F32, name="rstd")
                nc.vector.reciprocal(rstd[:], sd[:])
                mub = lnp.tile([128, 512], F32, name="pb")
                nc.tensor.matmul(mub[:], ones_f[:], mu[:], start=True, stop=True)
                rstdb = lnp.tile([128, 512], F32, name="pb")
                nc.tensor.matmul(rstdb[:], ones_f[:], rstd[:], start=True,
                                 stop=True)
                nbf = nbfp.tile([128, DT, 512], BF16, name="nbf")
                for dt_i in range(DT):
                    t1 = ev.tile([128, 512], F32, name="lntmp")
                    nc.vector.tensor_sub(t1[:], x_in[:, dt_i], mub[:])
                    t2 = ev.tile([128, 512], F32, name="lntmp")
                    nc.vector.tensor_mul(t2[:], t1[:], rstdb[:])
                    nc.scalar.activation(nbf[:, dt_i], t2[:],
                                         mybir.ActivationFunctionType.Identity,
                                         bias=b_t[:, dt_i:dt_i + 1],
                                         scale=s_t[:, dt_i:dt_i + 1])
            return nbf

        for l in range(NLx):
            slot = l % 2
            n1_bf = layernorm(xT, ln1s[l], ln1b[l])

            # ---- K^T, V projections (strips), then AllGather, then Q^T
            kt_sb = proj.tile([128, DT, 512], BF16, name="pA")
            v_sb = proj.tile([128, 4, H, 65], BF16, name="pA")
            qt_sb = proj.tile([128, DT, 512], BF16, name="pA")
            nc.vector.memset(v_sb[:, :, :, 64:65], 1.0)
            with tc.tile_pool(name="pqkv", bufs=4, space="PSUM") as pq_pool:
                wk_s = wpool.tile([128, DT, D], BF16, name="w")
                nc.sync.dma_start(wk_s[:], wk[l].rearrange("(a p) e -> p a e", p=128))
                for ft in range(DT):
                    pq = pq_pool.tile([128, 512], F32, name="pq")
                    for dt_i in range(DT):
                        nc.tensor.matmul(
                            pq[:], wk_s[:, dt_i, ft * 128:(ft + 1) * 128],
                            n1_bf[:, dt_i],
                            start=(dt_i == 0), stop=(dt_i == DT - 1))
                    nc.scalar.copy(kt_sb[:, ft], pq[:])
                kt_dr = agk_in[slot][:].rearrange("(p a t) -> p a t",
                                                  p=128, a=DT)
                nc.sync.dma_start(kt_dr, kt_sb[:])
                nc.gpsimd.collective_compute(
                    "AllGather", mybir.AluOpType.bypass, replica_groups=groups,
                    ins=[agk_in[slot][:]], outs=[agk_out[slot][:]],
                )

                wv_s = wpool.tile([128, DT, D], BF16, name="w")
                nc.scalar.dma_start(wv_s[:], wv[l].rearrange("(a p) e -> p a e", p=128))
                for half in range(2):
                    pvs = [pq_pool.tile([128, 512], F32, name="pq")
                           for _ in range(4)]
                    for dt_i in range(DT):
                        for tt in range(4):
                            nc.tensor.matmul(
                                pvs[tt][:],
                                n1_bf[:, dt_i, tt * 128:(tt + 1) * 128],
                                wv_s[:, dt_i, half * 512:(half + 1) * 512],
                                start=(dt_i == 0),
                                stop=(dt_i == DT - 1))
                    for tt in range(4):
                        nc.scalar.copy(
                            v_sb[:, tt, half * 8:(half + 1) * 8, 0:64],
                            pvs[tt][:].rearrange("p (a b) -> p a b", a=8))
                v_dr = agv_in[slot][:].rearrange("(p a t) -> p a t",
                                                 p=128, a=4)
                nc.sync.dma_start(v_dr, v_sb[:])
                nc.gpsimd.collective_compute(
                    "AllGather", mybir.AluOpType.bypass, replica_groups=groups,
                    ins=[agv_in[slot][:]], outs=[agv_out[slot][:]],
                )

                # Q projection overlaps the AllGather round-trip
                wq_s = wpool.tile([128, DT, D], BF16, name="w")
                nc.sync.dma_start(wq_s[:], wq[l].rearrange("(a p) e -> p a e", p=128))
                for ft in range(DT):
                    pq = pq_pool.tile([128, 512], F32, name="pq")
                    for dt_i in range(DT):
                        nc.tensor.matmul(
                            pq[:], wq_s[:, dt_i, ft * 128:(ft + 1) * 128],
                            n1_bf[:, dt_i],
                            start=(dt_i == 0), stop=(dt_i == DT - 1))
                    nc.scalar.copy(qt_sb[:, ft], pq[:])

            # prefetch Wo strip during attention
            wo_s = wpool.tile([128, DT, D], BF16, name="w")
            nc.scalar.dma_start(wo_s[:], wo[l].rearrange("(a p) d -> p a d", p=128))

            ktf = bigp.tile([128, 2, DT, 512], BF16, name="big")
            vf = bigp.tile([128, 2, 4, H, 65], BF16, name="big2")
            for blk in range(2):
                nc.sync.dma_start(
                    ktf[:, blk],
                    agk_out[slot][blk * KT_ELEMS:(blk + 1) * KT_ELEMS].rearrange(
                        "(p a t) -> p a t", p=128, a=DT))
                nc.sync.dma_start(
                    vf[:, blk],
                    agv_out[slot][blk * V_ELEMS:(blk + 1) * V_ELEMS].rearrange(
                        "(p a t) -> p a t", p=128, a=4))

            # ---- attention
            oT_all = proj.tile([128, DT, 512], BF16, name="pA")
            with tc.tile_pool(name="pst", bufs=3, space="PSUM") as pst_pool, \
                 tc.tile_pool(name="pacc", bufs=2, space="PSUM") as pacc, \
                 tc.tile_pool(name="pbc", bufs=2, space="PSUM") as pbc:
                for h in range(H):
                    po = h % 2 * 64
                    rt = h // 2
                    p_oT = pacc.tile([65, 512], F32, name="p_oT")
                    for kt in range(8):
                        blk = 1 - (kt % 2)
                        kslot = kt // 2
                        off = 128 * (kt // 2)
                        pst = pst_pool.tile([128, 512], F32, name="p_st")
                        nc.tensor.matmul(
                            pst[:, off:],
                            ktf[po:po + 64, blk, rt,
                                kslot * 128:(kslot + 1) * 128],
                            qt_sb[po:po + 64, rt, off:], start=True, stop=True)
                        st_bf = stp.tile([128, 512], BF16, name="st_bf")
                        nc.scalar.activation(st_bf[:, off:], pst[:, off:],
                                             mybir.ActivationFunctionType.Exp,
                                             scale=INV_SQRT_C)
                        nc.vector.tensor_mul(st_bf[:, off:off + 128],
                                             st_bf[:, off:off + 128],
                                             mask_t[:, kt % 2])
                        nc.tensor.matmul(p_oT[:, off:],
                                         vf[:, blk, kslot, h, :],
                                         st_bf[:, off:], start=(kt == 0),
                                         stop=(kt == 7))
                    recip = small.tile([1, 512], F32, name="recip")
                    nc.vector.reciprocal(recip[:], p_oT[64:65, :])
                    p_bc = pbc.tile([64, 512], F32, name="p_bc")
                    nc.tensor.matmul(p_bc[:], ones_f[:, :64], recip[:],
                                     start=True, stop=True)
                    bc_sb = ev.tile([64, 512], F32, name="bc_sb")
                    nc.scalar.copy(bc_sb[:], p_bc[:])
                    nc.vector.tensor_mul(oT_all[po:po + 64, rt], p_oT[0:64, :],
                                         bc_sb[:])

            # ---- Wo projection + residual + bo
            bo_t = small.tile([128, DT], F32, name="bo_t")
            nc.sync.dma_start(bo_t[:], bo_p[l])
            x2 = act32.tile([128, DT, 512], F32, name="xT")
            with tc.tile_pool(name="pwo", bufs=3, space="PSUM") as pwo:
                for dt_i in range(DT):
                    pw = pwo.tile([128, 512], F32, name="pw")
                    for et in range(DT):
                        nc.tensor.matmul(
                            pw[:], wo_s[:, et, dt_i * 128:(dt_i + 1) * 128],
                            oT_all[:, et],
                            start=(et == 0), stop=(et == DT - 1))
                    nc.vector.scalar_tensor_tensor(
                        x2[:, dt_i], pw[:], bo_t[:, dt_i:dt_i + 1], n1_bf[:, dt_i],
                        mybir.AluOpType.add, mybir.AluOpType.add)

            n2_bf = layernorm(x2, ln2s[l], ln2b[l])

            # ---- MLP in 8 hidden-octants with strip prefetch
            b1_t = small.tile([128, 32], F32, name="b1_t")
            nc.sync.dma_start(b1_t[:], b1_p[l])
            b2_t = small.tile([128, DT], F32, name="b2_t")
            nc.sync.dma_start(b2_t[:], b2_p[l])
            x3 = act32.tile([128, DT, 512], F32, name="xT")
            with tc.tile_pool(name="pmlp", bufs=3, space="PSUM") as pmlp:
                for oct_i in range(8):
                    w1o = mwp.tile([128, DT, 512], BF16, name="mw")
                    dma_eng = nc.sync if oct_i % 2 == 0 else nc.scalar
                    dma_eng.dma_start(
                        w1o[:],
                        w1[l, :, oct_i * 512:(oct_i + 1) * 512].rearrange(
                            "(a p) e -> p a e", p=128))
                    hTo = htp.tile([128, 4, 512], BF16, name="hTo")
                    for htl in range(4):
                        ph = pmlp.tile([128, 512], F32, name="ph")
                        for dt_i in range(DT):
                            nc.tensor.matmul(
                                ph[:], w1o[:, dt_i, htl * 128:(htl + 1) * 128],
                                n2_bf[:, dt_i],
                                start=(dt_i == 0), stop=(dt_i == DT - 1))
                        hidx = oct_i * 4 + htl
                        nc.scalar.activation(hTo[:, htl], ph[:],
                                             mybir.ActivationFunctionType.Relu,
                                             bias=b1_t[:, hidx:hidx + 1])
                    w2o = mwp.tile([128, 4, D], BF16, name="mw")
                    dma_eng2 = nc.scalar if oct_i % 2 == 0 else nc.sync
                    dma_eng2.dma_start(
                        w2o[:],
                        w2[l, oct_i * 512:(oct_i + 1) * 512, :].rearrange(
                            "(a p) e -> p a e", p=128))
                    for dt_i in range(DT):
                        py = pmlp.tile([128, 512], F32, name="py")
                        for htl in range(4):
                            nc.tensor.matmul(
                                py[:], w2o[:, htl, dt_i * 128:(dt_i + 1) * 128],
                                hTo[:, htl],
                                start=(htl == 0), stop=(htl == 3))
                        if oct_i == 0:
                            nc.vector.scalar_tensor_tensor(
                                x3[:, dt_i], py[:], b2_t[:, dt_i:dt_i + 1],
                                n2_bf[:, dt_i],
                                mybir.AluOpType.add, mybir.AluOpType.add)
                        else:
                            nc.vector.tensor_add(x3[:, dt_i], x3[:, dt_i], py[:])
            xT = x3

        # ---- final LN + LM head (50 vocab strips, bf16 out, bias on host)
        nf_bf = layernorm(xT, None, None)
        with tc.tile_pool(name="plm", bufs=4, space="PSUM") as plm:
            for c in range(NVC):
                lmS = wpool.tile([128, DT, 1024], BF16, name="w")
                dma_eng = nc.sync if c % 2 == 0 else nc.scalar
                dma_eng.dma_start(
                    lmS[:],
                    lmw[:, c * 1024:(c + 1) * 1024].rearrange(
                        "(a p) v -> p a v", p=128))
                for q_i in range(QT):
                    ot = otp.tile([128, 1024], BF16, name="ot")
                    for half in range(2):
                        pl = plm.tile([128, 512], F32, name="p_lm")
                        for dt_i in range(DT):
                            nc.tensor.matmul(
                                pl[:],
                                nf_bf[:, dt_i, q_i * 128:(q_i + 1) * 128],
                                lmS[:, dt_i, half * 512:(half + 1) * 512],
                                start=(dt_i == 0), stop=(dt_i == DT - 1))
                        nc.scalar.copy(ot[:, half * 512:(half + 1) * 512], pl[:])
                    nc.scalar.dma_start(
                        out[q_i * 128:(q_i + 1) * 128,
                            c * 1024:(c + 1) * 1024], ot[:])

    nc.compile()
    return nc

def host_prep(inputs, num_layers=L):
    """Build per-core in_maps + reassembly metadata from full inputs."""
    f32 = np.float32
    bf = ml_dtypes.bfloat16
    idx = np.asarray(inputs["idx"])
    tok_emb = np.asarray(inputs["tok_emb"], f32)
    pos_emb = np.asarray(inputs["pos_emb"], f32)

    def perD(a):  # [L?, D] -> [L?, 128, DT]
        a = np.asarray(a, f32)
        if a.ndim == 1:
            return np.ascontiguousarray(a.reshape(DT, 128).T)
        return np.ascontiguousarray(a.reshape(a.shape[0], -1, 128).transpose(0, 2, 1))

    wq = np.ascontiguousarray(np.asarray(inputs["Wq"], f32)[:num_layers]).astype(bf)
    wk = np.ascontiguousarray(np.asarray(inputs["Wk"], f32)[:num_layers]).astype(bf)
    wv = np.ascontiguousarray(np.asarray(inputs["Wv"], f32)[:num_layers]).astype(bf)
    wo = np.ascontiguousarray(np.asarray(inputs["Wo"], f32)[:num_layers]).astype(bf)
    w1 = np.ascontiguousarray(np.asarray(inputs["W1"], f32)[:num_layers]).astype(bf)
    w2 = np.ascontiguousarray(np.asarray(inputs["W2"], f32)[:num_layers]).astype(bf)
    ln1s = perD(inputs["ln1_s"])[:num_layers]
    ln1b = perD(inputs["ln1_b"])[:num_layers]
    ln2s = perD(inputs["ln2_s"])[:num_layers]
    ln2b = perD(inputs["ln2_b"])[:num_layers]
    bo_p = perD(inputs["bo"])[:num_layers]
    b1_p = perD(inputs["b1"])[:num_layers]
    b2_p = perD(inputs["b2"])[:num_layers]
    lnfs = perD(inputs["lnf_s"])
    lnfb = perD(inputs["lnf_b"])
    lmw = np.zeros((D, VPAD), f32)
    lmw[:, :V] = np.asarray(inputs["lm_W"], f32)
    lmw = lmw.astype(bf)
    lmb = np.asarray(inputs["lm_b"], f32)

    tri = np.tril(np.ones((128, 128), f32)).T  # mask[k, q] = 1 if k <= q
    m_ones = np.ones((128, 128), f32)
    m_zero = np.zeros((128, 128), f32)

    in_maps = []
    tiles_by_parity = []
    for c in range(8):
        b, p = c // 2, c % 2
        g_tiles = [2 * j + 1 - p for j in range(QT)]
        tiles_by_parity.append(g_tiles)
        rows = np.concatenate([np.arange(g * 128, (g + 1) * 128) for g in g_tiles])
        x0 = tok_emb[idx[b, rows]] + pos_emb[rows]          # [512, D]
        x0T = np.ascontiguousarray(
            x0.T.reshape(DT, 128, 512).transpose(1, 0, 2)).astype(f32)
        if p == 0:
            masks = np.stack([m_ones, tri])
        else:
            masks = np.stack([tri, m_zero])
        in_maps.append(dict(
            x0T=x0T, wq=wq, wk=wk, wv=wv, wo=wo, w1=w1, w2=w2,
            ln1s=ln1s, ln1b=ln1b, ln2s=ln2s, ln2b=ln2b,
            bo_p=bo_p, b1_p=b1_p, b2_p=b2_p, lnfs=lnfs, lnfb=lnfb,
            lmw=lmw, masks=masks.astype(bf),
        ))
    return in_maps, tiles_by_parity, lmb


def assemble(results, tiles_by_parity, lmb):
    out = np.empty((B, T, V), np.float32)
    bias = lmb[:V][None, :].astype(np.float32)
    for c in range(8):
        b = c // 2
        co = results[c]["out"]
        for j, g in enumerate(tiles_by_parity[c]):
            out[b, g * 128:(g + 1) * 128] = \
                np.asarray(co[j * 128:(j + 1) * 128, :V], np.float32) + bias
    return out


_CACHE = {}


def run(inputs, num_layers=L, trace=False):
    in_maps, tiles, lmb = host_prep(inputs, num_layers)
    key = num_layers
    if key not in _CACHE:
        _CACHE[key] = build_nc(num_layers)
    nc = _CACHE[key]
    res = run_bass_kernel_spmd(nc, in_maps, core_ids=list(range(8)), trace=trace)
    return assemble(res.results, tiles, lmb), res


def kernel(**inputs):
    out, _ = run(inputs, L)
    return out
